# revision 15
# baseline (speedup 1.0000x reference)
"""Conformer encoder (12-layer, B=8, T=512, D=512, H=8, DFF=2048, conv K=15)
on 8 Trainium2 NeuronCores.

Strategy: pure data-parallel — core b processes batch element b end-to-end.
The only cross-core coupling is the conv module's BatchNorm (batch statistics
over all B*T positions), handled with one tiny (1KB) AllReduce per layer.

On-core design (per core):
 - Activations kept feature-major (feature on partitions, time on free axis):
   chained matmuls then need NO transposes (weights as stored are [in, out] =
   exactly the lhsT layout the PE wants).
 - All matmuls in bf16 (weights pre-cast on host), f32 PSUM accumulation,
   f32 residual stream.
 - LayerNorm over the partition axis via ones-vector matmul reductions +
   K=1 matmul broadcast of the [1,T] stats.
 - Attention: scores computed transposed (tk on partitions, tq free) so the
   softmax denominator is a ones-matmul and A@V needs no transpose. The
   Transformer-XL legacy rel-shift is done exactly with a flat (T, T+1)
   DRAM buffer: bd_pre written row-major, re-read with an affine stride-T
   pattern, then PE-transposed and accumulated directly into the score PSUM.
 - Softmax without max subtraction (validated: |scores| < 20 for this model).
 - Depthwise conv K=15 as 15 accumulating PE matmuls with host-built
   diagonal weight matrices; BatchNorm stats via ACT accum_out +
   tensor_tensor_reduce, AllReduce'd across cores.
"""

import os
import sys

sys.path.insert(0, '/opt/trn_rl_repo')

import numpy as np

L, B, T, D, H, DFF, KC = 12, 8, 512, 512, 8, 2048, 15
DK = D // H
EPS = 1e-5
NCORES = 8
P = 128
NBT = float(B * T)

# vector slot table: name -> (offset, n_chunks of 128)
_SLOT_DEFS = [
    ('ln1g', 4), ('ln1b', 4), ('lnag', 4), ('lnab', 4), ('ln2g', 4), ('ln2b', 4),
    ('qub', 4), ('qvb', 4), ('bk', 4), ('bo', 4),
    ('ff1b1', 16), ('ff1b2h', 4), ('ff2b1', 16), ('ff2b2h', 4),
    ('pw1b', 8), ('dwb', 4), ('bng', 4), ('bnb', 4), ('pw2b', 4),
]
SLOT = {}
_off = 0
for _n, _c in _SLOT_DEFS:
    SLOT[_n] = _off
    _off += _c
NV = _off  # 104

_BUILT = {}


def _build(n_layers, stage='full'):
    import concourse.bass as bass
    import concourse.bacc as bacc
    import concourse.tile as tile
    from concourse import mybir
    from concourse.masks import make_identity

    f32 = mybir.dt.float32
    bf16 = mybir.dt.bfloat16
    AO = mybir.AluOpType
    AF = mybir.ActivationFunctionType

    nc = bacc.Bacc("TRN2", target_bir_lowering=False, debug=False,
                   num_devices=NCORES)

    # ---- parameters ----
    xT0 = nc.declare_dram_parameter("xT0", [D, T], bf16, isOutput=False)
    lin_w = nc.declare_dram_parameter("lin_w", [D, D], bf16, isOutput=False)
    vec_in = nc.declare_dram_parameter("vec_in", [P, 12], f32, isOutput=False)
    peT_d = nc.declare_dram_parameter("peT", [D, T], bf16, isOutput=False)
    NL = n_layers
    w_ff1w1 = nc.declare_dram_parameter("w_ff1w1", [NL, D, DFF], bf16, isOutput=False)
    w_ff1w2 = nc.declare_dram_parameter("w_ff1w2", [NL, DFF, D], bf16, isOutput=False)
    w_ff2w1 = nc.declare_dram_parameter("w_ff2w1", [NL, D, DFF], bf16, isOutput=False)
    w_ff2w2 = nc.declare_dram_parameter("w_ff2w2", [NL, DFF, D], bf16, isOutput=False)
    w_q = nc.declare_dram_parameter("w_q", [NL, D, D], bf16, isOutput=False)
    w_k = nc.declare_dram_parameter("w_k", [NL, D, D], bf16, isOutput=False)
    w_v = nc.declare_dram_parameter("w_v", [NL, D, D], bf16, isOutput=False)
    w_o = nc.declare_dram_parameter("w_o", [NL, D, D], bf16, isOutput=False)
    w_pos = nc.declare_dram_parameter("w_pos", [NL, D, D], bf16, isOutput=False)
    w_pw1 = nc.declare_dram_parameter("w_pw1", [NL, D, 2 * D], bf16, isOutput=False)
    w_pw2 = nc.declare_dram_parameter("w_pw2", [NL, D, D], bf16, isOutput=False)
    w_diag = nc.declare_dram_parameter("w_diag", [NL, 4, P, KC * P], bf16, isOutput=False)
    vecs_d = nc.declare_dram_parameter("vecs", [NL, P, NV], f32, isOutput=False)
    bvrow_d = nc.declare_dram_parameter("bvrow", [NL, 1, D], bf16, isOutput=False)
    out_d = nc.declare_dram_parameter("out", [D, T], f32, isOutput=True)

    XP = T * (T + 1)

    with tile.TileContext(nc) as tc:
        from contextlib import ExitStack
        ctx = ExitStack()
        with ctx:
            sb = ctx.enter_context(tc.tile_pool(name="sb", bufs=1))
            ps = ctx.enter_context(tc.tile_pool(name="ps", bufs=1, space="PSUM"))
            dram = ctx.enter_context(tc.tile_pool(name="dram", bufs=1, space="DRAM"))

            def st(shape, dtype, tag, bufs):
                return sb.tile(shape, dtype, tag=tag, bufs=bufs, name=tag)

            def pst(shape, dtype, tag, bufs):
                return ps.tile(shape, dtype, tag=tag, bufs=bufs, name=tag)

            # ---- constants ----
            ones = st([P, 1], bf16, "ones", 1)
            nc.vector.memset(ones, 1.0)
            onesk1 = st([1, P], bf16, "onesk1", 1)
            nc.vector.memset(onesk1, 1.0)
            ident = st([P, P], bf16, "ident", 1)
            make_identity(nc, ident)
            eps1 = st([1, 1], f32, "eps1", 1)
            nc.vector.memset(eps1, EPS)
            epsP = st([P, 1], f32, "epsP", 1)
            nc.vector.memset(epsP, EPS)
            zrow = st([P, 4], bf16, "zrow", 1)
            nc.vector.memset(zrow, 0.0)
            vin = st([P, 12], f32, "vin", 1)
            nc.sync.dma_start(out=vin, in_=vec_in[:, :])
            peT = [st([P, T], bf16, f"peT{c}", 1) for c in range(4)]
            for c in range(4):
                nc.sync.dma_start(out=peT[c], in_=peT_d[c * P:(c + 1) * P, :])
            x = [st([P, T], f32, f"x{c}", 1) for c in range(4)]

            # rel-shift scratch buffers (flat (T, T+1) row-major), zero col 0
            xp = [dram.tile([XP], bf16, tag=f"xp{i}", name=f"xp{i}") for i in range(2)]
            for i in range(2):
                zap = bass.AP(tensor=xp[i].tensor, offset=xp[i].offset,
                              ap=[[T + 1, P], [(T + 1) * P, 4]])
                nc.sync.dma_start(out=zap, in_=zrow)

            def ln(in_tiles, gslot, bslot, out_dtype, vec):
                """LayerNorm over partitions (feature axis). Returns 4 tiles."""
                xbf, sq = [], []
                for c in range(4):
                    t = st([P, T], bf16, "lncast", 4)
                    nc.vector.tensor_copy(t, in_tiles[c])
                    xbf.append(t)
                    t2 = st([P, T], bf16, "lnsq", 3)
                    nc.scalar.square(t2, in_tiles[c])
                    sq.append(t2)
                ps_sq = pst([33, T], f32, "psrow", 2)
                ps_s, ps_q = ps_sq[0:1, :], ps_sq[32:33, :]
                for c in range(4):
                    nc.tensor.matmul(ps_s, ones, xbf[c], start=(c == 0), stop=(c == 3))
                for c in range(4):
                    nc.tensor.matmul(ps_q, ones, sq[c], start=(c == 0), stop=(c == 3))
                m_row = st([1, T], f32, "lnm", 2)
                nc.vector.tensor_scalar(m_row, ps_s, 1.0 / D, None, AO.mult)
                var = st([1, T], f32, "lnvar", 2)
                nc.vector.tensor_scalar(var, ps_q, 1.0 / D, None, AO.mult)
                m2 = st([1, T], f32, "lnm2", 2)
                nc.vector.tensor_tensor(m2, m_row, m_row, AO.mult)
                nc.vector.tensor_tensor(var, var, m2, AO.subtract)
                nc.scalar.activation(var, var, AF.Sqrt, bias=eps1)
                r_row = st([1, T], f32, "lnr", 2)
                nc.vector.reciprocal(r_row, var)
                mbf = st([1, T], bf16, "lnmbf", 2)
                nc.vector.tensor_copy(mbf, m_row)
                rbf = st([1, T], bf16, "lnrbf", 2)
                nc.vector.tensor_copy(rbf, r_row)
                ps_mb = pst([P, T], f32, "psbc", 2)
                nc.tensor.matmul(ps_mb, onesk1, mbf, start=True, stop=True)
                ps_rb = pst([P, T], f32, "psbc", 2)
                nc.tensor.matmul(ps_rb, onesk1, rbf, start=True, stop=True)
                outs = []
                for c in range(4):
                    t1 = st([P, T], f32, "lnt", 3)
                    nc.vector.tensor_tensor(t1, in_tiles[c], ps_mb, AO.subtract)
                    nc.vector.tensor_tensor(t1, t1, ps_rb, AO.mult)
                    o = st([P, T], out_dtype, "lnout", 6)
                    nc.vector.tensor_scalar(
                        o, t1, vec[:, gslot + c:gslot + c + 1],
                        vec[:, bslot + c:bslot + c + 1], AO.mult, AO.add)
                    outs.append(o)
                return outs

            def ffn(vec, w1_l, w2_l, g0, b0, b1s, b2s):
                nx = ln(x, g0, b0, bf16, vec)
                w1t = []
                for c in range(4):
                    t = st([P, DFF], bf16, "w1", 4)
                    nc.sync.dma_start(
                        out=t, in_=w1_l.rearrange("(c p) m -> c p m", p=P)[c])
                    w1t.append(t)
                h = []
                for m in range(16):
                    pp = pst([P, T], f32, "mm", 4)
                    for c in range(4):
                        nc.tensor.matmul(pp, w1t[c][:, m * P:(m + 1) * P], nx[c],
                                         start=(c == 0), stop=(c == 3))
                    ht = st([P, T], bf16, "h", 16)
                    nc.scalar.activation(ht, pp, AF.Relu,
                                         bias=vec[:, b1s + m:b1s + m + 1])
                    h.append(ht)
                # w2: k-outer so weight tiles stream with 4 bufs
                pps = [pst([P, T], f32, "mm", 4) for _ in range(4)]
                for k in range(16):
                    t = st([P, D], bf16, "w2", 4)
                    nc.sync.dma_start(
                        out=t, in_=w2_l.rearrange("(k p) m -> k p m", p=P)[k])
                    for m in range(4):
                        nc.tensor.matmul(pps[m], t[:, m * P:(m + 1) * P], h[k],
                                         start=(k == 0), stop=(k == 15))
                for m in range(4):
                    t = st([P, T], f32, "res", 4)
                    nc.scalar.activation(t, pps[m], AF.Identity,
                                         bias=vec[:, b2s + m:b2s + m + 1])
                    nc.vector.tensor_tensor(x[m], x[m], t, AO.add)

            def attention(li, vec, use_bd=True):
                nx = ln(x, SLOT['lnag'], SLOT['lnab'], bf16, vec)
                awt = {}
                for nm, wd in (('q', w_q), ('k', w_k), ('v', w_v), ('o', w_o),
                               ('p', w_pos)):
                    tl = []
                    for c in range(4):
                        t = st([P, D], bf16, "aw", 8)
                        nc.sync.dma_start(
                            out=t,
                            in_=wd[li].rearrange("(c p) m -> c p m", p=P)[c])
                        tl.append(t)
                    awt[nm] = tl
                bvr = st([1, D], bf16, "bv", 2)
                nc.sync.dma_start(out=bvr, in_=bvrow_d[li])
                qu, qv, kT, vtm, pT = [], [], [], [], []
                for m in range(4):
                    pp = pst([P, T], f32, "mm", 4)
                    for c in range(4):
                        nc.tensor.matmul(pp, awt['q'][c][:, m * P:(m + 1) * P],
                                         nx[c], start=(c == 0), stop=(c == 3))
                    a = st([P, T], bf16, "qkv", 24)
                    nc.vector.tensor_scalar(
                        a, pp, vec[:, SLOT['qub'] + m:SLOT['qub'] + m + 1], None,
                        AO.add)
                    qu.append(a)
                    b = st([P, T], bf16, "qkv", 24)
                    nc.vector.tensor_scalar(
                        b, pp, vec[:, SLOT['qvb'] + m:SLOT['qvb'] + m + 1], None,
                        AO.add)
                    qv.append(b)
                for m in range(4):
                    pp = pst([P, T], f32, "mm", 4)
                    for c in range(4):
                        nc.tensor.matmul(pp, awt['k'][c][:, m * P:(m + 1) * P],
                                         nx[c], start=(c == 0), stop=(c == 3))
                    a = st([P, T], bf16, "qkv", 24)
                    nc.vector.tensor_scalar(
                        a, pp, vec[:, SLOT['bk'] + m:SLOT['bk'] + m + 1], None,
                        AO.add)
                    kT.append(a)
                # V time-major (+bias via K=1 ones matmul)
                for tt in range(4):
                    pp = pst([P, T], f32, "mm", 4)
                    nc.tensor.matmul(pp, onesk1, bvr, start=True, stop=False,
                                     skip_group_check=True)
                    for c in range(4):
                        nc.tensor.matmul(pp, nx[c][:, tt * P:(tt + 1) * P],
                                         awt['v'][c], start=False, stop=(c == 3),
                                         skip_group_check=True)
                    a = st([P, T], bf16, "qkv", 24)
                    nc.vector.tensor_copy(a, pp)
                    vtm.append(a)
                for m in range(4):
                    pp = pst([P, T], f32, "mm", 4)
                    for c in range(4):
                        nc.tensor.matmul(pp, awt['p'][c][:, m * P:(m + 1) * P],
                                         peT[c], start=(c == 0), stop=(c == 3))
                    a = st([P, T], bf16, "qkv", 24)
                    nc.vector.tensor_copy(a, pp)
                    pT.append(a)
                oT = [st([P, T], bf16, "qkv", 24) for _ in range(4)]
                if stage == 'attnproj':
                    for c in range(4):
                        nc.vector.tensor_copy(oT[c], qu[c])
                for hh in range(H if stage not in ('attnproj',) else 0):
                    c, r0 = hh // 2, 64 * (hh % 2)
                    qu_h = qu[c][r0:r0 + 64, :]
                    qv_h = qv[c][r0:r0 + 64, :]
                    k_h = kT[c][r0:r0 + 64, :]
                    p_h_ = pT[c][r0:r0 + 64, :]
                    xpb = xp[hh % 2]
                    # bd_pre (q-major) -> DRAM flat buffer
                    for i in range(4 if use_bd else 0):
                        pp = pst([P, T], f32, "mm", 4)
                        nc.tensor.matmul(pp, qv_h[:, i * P:(i + 1) * P], p_h_,
                                         start=True, stop=True)
                        bt = st([P, T], bf16, "bdw", 4)
                        nc.vector.tensor_copy(bt, pp)
                        wap = bass.AP(tensor=xpb.tensor,
                                      offset=xpb.offset + i * P * (T + 1) + 1,
                                      ap=[[T + 1, P], [1, T]])
                        nc.sync.dma_start(out=wap, in_=bt)
                    # shifted read-back (q-major, contiguous)
                    bdr = []
                    for i in range(4 if use_bd else 0):
                        bt = st([P, T], bf16, "bdr", 6)
                        rap = bass.AP(tensor=xpb.tensor,
                                      offset=xpb.offset + T + i * P * T,
                                      ap=[[T, P], [1, T]])
                        nc.sync.dma_start(out=bt, in_=rap)
                        bdr.append(bt)
                    # scores (q-major): ac matmul, += bd, exp (+row sums), 1/Z
                    attn_n = []
                    for i in range(4 if stage != 'attnsc' else 1):
                        pp = pst([P, T], f32, "mm", 4)
                        nc.tensor.matmul(pp, qu_h[:, i * P:(i + 1) * P], k_h,
                                         start=True, stop=True)
                        if use_bd:
                            nc.vector.tensor_tensor(pp, pp, bdr[i], AO.add)
                        et = st([P, T], bf16, "et", 2)
                        zcol = st([P, 1], f32, "zcol", 4)
                        nc.scalar.activation(et, pp, AF.Exp, scale=0.125,
                                             accum_out=zcol)
                        rz = st([P, 1], f32, "rz", 4)
                        nc.vector.reciprocal(rz, zcol)
                        at = st([P, T], bf16, "attn", 4)
                        nc.vector.tensor_scalar(at, et, rz, None, AO.mult)
                        attn_n.append(at)
                    if stage == 'attnsc':
                        nc.vector.tensor_copy(oT[c][r0:r0 + 64, :],
                                              attn_n[0][0:64, :])
                        continue
                    # transpose normalized attention to tk-major (bf16 PSUM)
                    etT = []
                    for j in range(4):
                        pt_ = pst([P, T], bf16, "mm", 4)
                        for i in range(4):
                            nc.tensor.matmul(
                                pt_[:, i * P:(i + 1) * P],
                                attn_n[i][:, j * P:(j + 1) * P], ident,
                                is_transpose=True, start=True, stop=True,
                                skip_group_check=True)
                        tt_ = st([P, T], bf16, "etT", 3)
                        nc.vector.tensor_copy(tt_, pt_)
                        etT.append(tt_)
                    if stage == 'attntr':
                        nc.vector.tensor_copy(oT[c][r0:r0 + 64, :],
                                              etT[0][0:64, :])
                        continue
                    # A @ V (output written at the head's partition offset)
                    psav = pst([P, T], f32, "mm", 4)
                    for j in range(4):
                        nc.tensor.matmul(psav[r0:r0 + 64, :],
                                         vtm[j][:, hh * 64:hh * 64 + 64],
                                         etT[j], start=(j == 0), stop=(j == 3))
                    nc.vector.tensor_copy(oT[c][r0:r0 + 64, :],
                                          psav[r0:r0 + 64, :])
                # output projection + residual
                for m in range(4):
                    pp = pst([P, T], f32, "mm", 4)
                    for c in range(4):
                        nc.tensor.matmul(pp, awt['o'][c][:, m * P:(m + 1) * P],
                                         oT[c], start=(c == 0), stop=(c == 3))
                    t = st([P, T], f32, "res", 4)
                    nc.scalar.activation(
                        t, pp, AF.Identity,
                        bias=vec[:, SLOT['bo'] + m:SLOT['bo'] + m + 1])
                    nc.vector.tensor_tensor(x[m], x[m], t, AO.add)

            def conv(li, vec, sub='full'):
                nx = ln(x, SLOT['lnag'], SLOT['lnab'], bf16, vec)
                pw1t = []
                for c in range(4):
                    t = st([P, 2 * D], bf16, "pw1", 4)
                    nc.sync.dma_start(
                        out=t, in_=w_pw1[li].rearrange("(c p) m -> c p m", p=P)[c])
                    pw1t.append(t)
                # pointwise conv1 + GLU into padded conv-input tiles
                a_t = []
                for m in range(4):
                    pp = pst([P, T], f32, "mm", 4)
                    for c in range(4):
                        nc.tensor.matmul(pp, pw1t[c][:, m * P:(m + 1) * P],
                                         nx[c], start=(c == 0), stop=(c == 3))
                    t = st([P, T], bf16, "glua", 4)
                    nc.scalar.activation(
                        t, pp, AF.Identity,
                        bias=vec[:, SLOT['pw1b'] + m:SLOT['pw1b'] + m + 1])
                    a_t.append(t)
                glu = []
                for m in range(4):
                    pp = pst([P, T], f32, "mm", 4)
                    for c in range(4):
                        nc.tensor.matmul(pp, pw1t[c][:, (4 + m) * P:(5 + m) * P],
                                         nx[c], start=(c == 0), stop=(c == 3))
                    sg = st([P, T], bf16, "sig", 2)
                    nc.scalar.activation(
                        sg, pp, AF.Sigmoid,
                        bias=vec[:, SLOT['pw1b'] + 4 + m:SLOT['pw1b'] + 5 + m])
                    gp = st([P, T + KC - 1], bf16, "glu", 4)
                    nc.vector.memset(gp[:, 0:7], 0.0)
                    nc.vector.memset(gp[:, T + 7:T + 14], 0.0)
                    nc.vector.tensor_tensor(gp[:, 7:7 + T], a_t[m], sg, AO.mult)
                    glu.append(gp)
                if sub == 'glu':
                    for m in range(4):
                        nc.vector.tensor_tensor(x[m], x[m], a_t[m], AO.add)
                    return
                # depthwise conv as 15 accumulating diagonal matmuls
                hc, stat = [], []
                for c in range(4):
                    dg = st([P, KC * P], bf16, "diag", 2)
                    nc.sync.dma_start(out=dg, in_=w_diag[li, c])
                    pp = pst([P, T], f32, "mm", 4)
                    for j in range(KC):
                        nc.tensor.matmul(pp, dg[:, j * P:(j + 1) * P],
                                         glu[c][:, j:j + T], start=(j == 0),
                                         stop=(j == KC - 1))
                    stt = st([P, 2], f32, "stat", 8)
                    hh = st([P, T], f32, "hc", 4)
                    nc.scalar.activation(
                        hh, pp, AF.Identity,
                        bias=vec[:, SLOT['dwb'] + c:SLOT['dwb'] + c + 1],
                        accum_out=stt[:, 0:1])
                    scr = st([P, T], f32, "scr", 1)
                    nc.scalar.square(scr, hh)
                    nc.vector.tensor_reduce(stt[:, 1:2], scr,
                                            axis=mybir.AxisListType.X, op=AO.add)
                    hc.append(hh)
                    stat.append(stt)
                if sub == 'dw':
                    for m in range(4):
                        nc.vector.tensor_tensor(x[m], x[m], hc[m], AO.add)
                    return
                # cross-core BN stats
                bn_in = dram.tile([2 * D], f32, tag="bnin", name="bnin", bufs=2)
                bn_out = dram.tile([2 * D], f32, tag="bnout", name="bnout", bufs=2)
                for c in range(4):
                    oap = bass.AP(tensor=bn_in.tensor,
                                  offset=bn_in.offset + c * 2 * P,
                                  ap=[[2, P], [1, 2]])
                    nc.sync.dma_start(out=oap, in_=stat[c])
                nc.gpsimd.collective_compute(
                    "AllReduce", AO.add,
                    replica_groups=[list(range(NCORES))],
                    ins=[bn_in.opt()], outs=[bn_out.opt()])
                sw = []
                for c in range(4):
                    gst = st([P, 2], f32, "gstat", 8)
                    iap = bass.AP(tensor=bn_out.tensor,
                                  offset=bn_out.offset + c * 2 * P,
                                  ap=[[2, P], [1, 2]])
                    nc.sync.dma_start(out=gst, in_=iap)
                    mt = st([P, 1], f32, "bnm", 4)
                    nc.vector.tensor_scalar(mt, gst[:, 0:1], 1.0 / NBT, None,
                                            AO.mult)
                    var = st([P, 1], f32, "bnvar", 4)
                    nc.vector.tensor_scalar(var, gst[:, 1:2], 1.0 / NBT, None,
                                            AO.mult)
                    m2 = st([P, 1], f32, "bnm2", 4)
                    nc.vector.tensor_tensor(m2, mt, mt, AO.mult)
                    nc.vector.tensor_tensor(var, var, m2, AO.subtract)
                    nc.scalar.activation(var, var, AF.Sqrt, bias=epsP)
                    rs = st([P, 1], f32, "bnrs", 4)
                    nc.vector.reciprocal(rs, var)
                    sc = st([P, 1], f32, "bnsc", 4)
                    nc.vector.tensor_tensor(
                        sc, rs, vec[:, SLOT['bng'] + c:SLOT['bng'] + c + 1],
                        AO.mult)
                    bi = st([P, 1], f32, "bnbi", 4)
                    nc.vector.tensor_tensor(bi, mt, sc, AO.mult)
                    nc.vector.tensor_tensor(
                        bi, vec[:, SLOT['bnb'] + c:SLOT['bnb'] + c + 1], bi,
                        AO.subtract)
                    hn = st([P, T], f32, "hn", 2)
                    nc.vector.tensor_scalar(hn, hc[c], sc, bi, AO.mult, AO.add)
                    sg2 = st([P, T], bf16, "sg2", 2)
                    nc.scalar.activation(sg2, hn, AF.Sigmoid)
                    swt = st([P, T], bf16, "swt", 4)
                    nc.vector.tensor_tensor(swt, hn, sg2, AO.mult)
                    sw.append(swt)
                if sub == 'bn':
                    for m in range(4):
                        nc.vector.tensor_tensor(x[m], x[m], sw[m], AO.add)
                    return
                pw2t = []
                for c in range(4):
                    t = st([P, D], bf16, "pw2", 4)
                    nc.sync.dma_start(
                        out=t, in_=w_pw2[li].rearrange("(c p) m -> c p m", p=P)[c])
                    pw2t.append(t)
                for m in range(4):
                    pp = pst([P, T], f32, "mm", 4)
                    for c in range(4):
                        nc.tensor.matmul(pp, pw2t[c][:, m * P:(m + 1) * P],
                                         sw[c], start=(c == 0), stop=(c == 3))
                    t = st([P, T], f32, "res", 4)
                    nc.scalar.activation(
                        t, pp, AF.Identity,
                        bias=vec[:, SLOT['pw2b'] + m:SLOT['pw2b'] + m + 1])
                    nc.vector.tensor_tensor(x[m], x[m], t, AO.add)

            # ---- input projection + input LN ----
            x0sb = []
            for c in range(4):
                t = st([P, T], bf16, "lncast", 4)
                nc.sync.dma_start(out=t, in_=xT0[c * P:(c + 1) * P, :])
                x0sb.append(t)
            lwt = []
            for c in range(4):
                t = st([P, D], bf16, "aw", 8)
                nc.sync.dma_start(
                    out=t, in_=lin_w.rearrange("(c p) m -> c p m", p=P)[c])
                lwt.append(t)
            xraw = []
            for m in range(4):
                pp = pst([P, T], f32, "mm", 4)
                for c in range(4):
                    nc.tensor.matmul(pp, lwt[c][:, m * P:(m + 1) * P], x0sb[c],
                                     start=(c == 0), stop=(c == 3))
                t = st([P, T], f32, "res", 4)
                nc.scalar.activation(t, pp, AF.Identity, bias=vin[:, m:m + 1])
                xraw.append(t)
            # input LN writes the residual stream x (f32)
            nx0 = ln(xraw, 4, 8, f32, vin)
            for c in range(4):
                nc.vector.tensor_copy(x[c], nx0[c])

            # ---- layers ----
            for li in range(NL if stage != 'inproj' else 0):
                vec = st([P, NV], f32, "vec", 2)
                nc.sync.dma_start(out=vec, in_=vecs_d[li])
                ffn(vec, w_ff1w1[li], w_ff1w2[li], SLOT['ln1g'], SLOT['ln1b'],
                    SLOT['ff1b1'], SLOT['ff1b2h'])
                if stage in ('attnproj', 'attnsc', 'attntr', 'attn_nobd', 'attn', 'conv', 'full'):
                    attention(li, vec, use_bd=(stage not in ('attn_nobd', 'attnsc', 'attntr', 'attnproj')))
                if stage in ('convglu', 'convdw', 'convbn', 'conv', 'full'):
                    sub = {'convglu': 'glu', 'convdw': 'dw', 'convbn': 'bn'}.get(stage, 'full')
                    conv(li, vec, sub=sub)
                if stage == 'full':
                    ffn(vec, w_ff2w1[li], w_ff2w2[li], SLOT['ln2g'], SLOT['ln2b'],
                        SLOT['ff2b1'], SLOT['ff2b2h'])

            # ---- output ----
            for c in range(4):
                nc.sync.dma_start(out=out_d[c * P:(c + 1) * P, :], in_=x[c])

    nc.compile()
    return nc


def _chunks(v, n):
    return [v[i * P:(i + 1) * P] for i in range(n)]


def _prep(inputs, n_layers):
    """Host-side preprocessing: layouts + bf16 casts. Returns (shared_map, per_core)."""
    from concourse import mybir
    bfnp = mybir.dt.np(mybir.dt.bfloat16)
    f32 = np.float32

    inp = {k: np.asarray(v, dtype=np.float32) for k, v in inputs.items()}
    NL = n_layers

    def bf(a):
        return np.ascontiguousarray(a).astype(bfnp)

    shared = {}
    shared['lin_w'] = bf(inp['lin_in_w'])
    # pe table (constant)
    pos = np.arange(T, dtype=f32)[:, None]
    div = np.exp(np.arange(0, D, 2, dtype=f32) * (-np.log(10000.0) / D))
    pe = np.zeros((T, D), f32)
    pe[:, 0::2] = np.sin(pos * div)
    pe[:, 1::2] = np.cos(pos * div)
    shared['peT'] = bf(pe.T)
    vin = np.stack(_chunks(inp['lin_in_b'], 4) + _chunks(inp['ln_in_g'], 4)
                   + _chunks(inp['ln_in_b'], 4))  # (12, 128)
    shared['vec_in'] = np.ascontiguousarray(vin.T)
    shared['w_ff1w1'] = bf(inp['ff1_w1'][:NL])
    shared['w_ff1w2'] = bf(0.5 * inp['ff1_w2'][:NL])
    shared['w_ff2w1'] = bf(inp['ff2_w1'][:NL])
    shared['w_ff2w2'] = bf(0.5 * inp['ff2_w2'][:NL])
    for nm, key in (('w_q', 'wq'), ('w_k', 'wk'), ('w_v', 'wv'), ('w_o', 'wo'),
                    ('w_pos', 'wpos')):
        shared[nm] = bf(inp[key][:NL])
    shared['w_pw1'] = bf(inp['pw1_w'][:NL].transpose(0, 2, 1))
    shared['w_pw2'] = bf(inp['pw2_w'][:NL].transpose(0, 2, 1))
    dd = np.zeros((NL, 4, P, KC, P), f32)
    idx = np.arange(P)
    dd[:, :, idx, :, idx] = inp['dw_w'][:NL].reshape(NL, 4, P, KC).transpose(2, 0, 1, 3)
    shared['w_diag'] = bf(dd.reshape(NL, 4, P, KC * P))
    shared['bvrow'] = bf(inp['bv'][:NL].reshape(NL, 1, D))

    vecs = np.zeros((NL, NV, P), f32)
    for li in range(NL):
        def put(name, v):
            off = SLOT[name]
            for c, ch in enumerate(_chunks(v, len(v) // P)):
                vecs[li, off + c] = ch
        put('ln1g', inp['ln_ff1_g'][li]); put('ln1b', inp['ln_ff1_b'][li])
        put('lnag', inp['ln_attn_g'][li]); put('lnab', inp['ln_attn_b'][li])
        put('ln2g', inp['ln_ff2_g'][li]); put('ln2b', inp['ln_ff2_b'][li])
        put('qub', inp['bq'][li] + inp['pbu'][li].reshape(D))
        put('qvb', inp['bq'][li] + inp['pbv'][li].reshape(D))
        put('bk', inp['bk'][li]); put('bo', inp['bo'][li])
        put('ff1b1', inp['ff1_b1'][li]); put('ff1b2h', 0.5 * inp['ff1_b2'][li])
        put('ff2b1', inp['ff2_b1'][li]); put('ff2b2h', 0.5 * inp['ff2_b2'][li])
        put('pw1b', inp['pw1_b'][li]); put('dwb', inp['dw_b'][li])
        put('bng', inp['bn_g'][li]); put('bnb', inp['bn_b'][li])
        put('pw2b', inp['pw2_b'][li])
    shared['vecs'] = np.ascontiguousarray(vecs.transpose(0, 2, 1))

    per_core = []
    xT0 = inp['padded_input'].transpose(0, 2, 1)  # (B, D, T)
    for b in range(NCORES):
        per_core.append({'xT0': bf(xT0[b])})
    return shared, per_core


def kernel(**inputs):
    from concourse.bass_utils import run_bass_kernel_spmd

    n_layers = int(os.environ.get('KERNEL_LAYERS', str(L)))
    stage = os.environ.get('KERNEL_STAGE', 'full')
    key = (n_layers, stage)
    if key not in _BUILT:
        _BUILT[key] = _build(n_layers, stage)
    nc = _BUILT[key]

    shared, per_core = _prep(inputs, n_layers)
    in_maps = [dict(shared, **per_core[b]) for b in range(NCORES)]
    trace = os.environ.get('KERNEL_TRACE', '0') == '1'
    if trace:
        sys.path.insert(0, '/root/problem/work')
        try:
            import hookinj
            hookinj.install()
        except Exception:
            pass
    res = run_bass_kernel_spmd(nc, in_maps, list(range(NCORES)), trace=trace)
    kernel.last_exec_time_ns = res.exec_time_ns
    out = np.stack([res.results[b]['out'].T for b in range(NCORES)])
    return out.astype(np.float32)


kernel.last_exec_time_ns = None
